# revision 4
# baseline (speedup 1.0000x reference)
"""Caser (conv seq-rec) forward pass on 8 Trainium2 NeuronCores.

Sharding: data-parallel over batch (B=512 -> 64/core) for the conv/fc
path; tensor-parallel over the 50k item vocab for fc_predict (Wp/bp/
logit, 6250 rows/core).  seq_output is AllGather-ed on device between
the two phases so everything runs in a single NEFF.

Layout convention on device: features on SBUF partitions, batch on the
free dimension.  All weights are pre-transposed/packed on host so every
DMA is partition-major and every matmul is a plain [K,M]x[K,N] with
K=128.

Algebraic rewrite: the vertical conv (Wv) is folded into the first FC
layer: z = relu(x . Mfold + out_h @ W1h^T + b1') with
Mfold[k,l,d] = sum_c W1[k, c*D+d] * Wv[c,l] and
b1'[k] = b1[k] + sum_c bv[c] * sum_d W1v[k,c,d].

Horizontal convs (heights i=1..31) are computed as shift-matmuls
accumulated in PSUM, two heights (2k-1, 2k) packed into the 128 output
partitions per pass; relu+maxpool commute (relu monotone) so pooling is
a PSUM reduce_max followed by a biased relu.
"""

import numpy as np

B, L, D, H, V, NH, NV, NU = 512, 31, 256, 512, 50000, 64, 64, 10000
NCORES = 8
BL = B // NCORES            # 64 batches per core
VS = V // NCORES            # 6250 vocab rows per core
VCH = (VS + 127) // 128     # 49 vocab chunks of 128
VPAD = VCH * 128            # 6272

_CACHE = {}
_LAST_IN_MAPS = None


def _build_program():
    import concourse.mybir as mybir
    import concourse.tile as tile
    from concourse import bacc

    f32 = mybir.dt.float32
    Relu = mybir.ActivationFunctionType.Relu
    Ident = mybir.ActivationFunctionType.Identity
    X = mybir.AxisListType.X

    nc = bacc.Bacc("TRN2", target_bir_lowering=False, debug=False,
                   num_devices=NCORES)

    def din(name, shape):
        return nc.dram_tensor(name, list(shape), f32, kind="ExternalInput").ap()

    def dout(name, shape):
        return nc.dram_tensor(name, list(shape), f32, kind="ExternalOutput").ap()

    xT_d = din("xT", (2, 128, BL, 32))
    whm_d = din("whm", (128, 240, 2, 128))
    whs_d = din("whs", (128, 31, 2, 64))
    bh_d = din("bh_t", (128, 16))
    mf_d = din("mfl", (128, 31, 2, 2, 128))
    b1_d = din("b1p", (128, 2))
    w1h_d = din("w1ht", (128, 16, 2, 128))
    ue_d = din("ueT", (128, 2, BL))
    w2_d = din("w2t", (128, 4, 4, 128))
    b2_d = din("b2r", (128, 4))
    wsc_d = din("wsct", (128, 2, 4, 128))
    bsc_d = din("bsc_t", (128, 4))
    wp_d = din("wpt", (128, 4, VPAD))
    bp_d = din("bp_t", (128, VCH))

    s_out_d = dout("s_out", (128, 4, BL))
    hs_out_d = dout("hseq_out", (128, 4, BL, L))
    lg_out_d = dout("logit_out", (VCH, 128, 512))

    with tile.TileContext(nc) as tc:
        with tc.tile_pool(name="const", bufs=1) as constp, \
             tc.tile_pool(name="wtp", bufs=2) as wtp, \
             tc.tile_pool(name="mfp", bufs=2) as mfp, \
             tc.tile_pool(name="wpp", bufs=3) as wpp, \
             tc.tile_pool(name="tmpp", bufs=4) as tmpp, \
             tc.tile_pool(name="evacp", bufs=3) as evacp, \
             tc.tile_pool(name="psconv", bufs=4, space="PSUM") as psconv, \
             tc.tile_pool(name="psfc", bufs=2, space="PSUM") as psfc, \
             tc.tile_pool(name="psout", bufs=2, space="PSUM") as psout, \
             tc.tile_pool(name="dram", bufs=1, space="DRAM") as dramp:

            xt = constp.tile([128, 2, BL, 32], f32)
            for dc in range(2):
                nc.sync.dma_start(xt[:, dc], xT_d[dc])

            bh_t = constp.tile([128, 16], f32)
            nc.sync.dma_start(bh_t[:], bh_d[:])
            b1_t = constp.tile([128, 2], f32)
            nc.sync.dma_start(b1_t[:], b1_d[:])
            b2_t = constp.tile([128, 4], f32)
            nc.sync.dma_start(b2_t[:], b2_d[:])
            bsc_t = constp.tile([128, 4], f32)
            nc.sync.dma_start(bsc_t[:], bsc_d[:])
            bp_t = constp.tile([128, VCH], f32)
            nc.sync.dma_start(bp_t[:], bp_d[:])
            w1h = constp.tile([128, 16, 2, 128], f32)
            nc.sync.dma_start(w1h[:], w1h_d[:])
            w2 = constp.tile([128, 4, 4, 128], f32)
            nc.sync.dma_start(w2[:], w2_d[:])
            wsc = constp.tile([128, 2, 4, 128], f32)
            nc.sync.dma_start(wsc[:], wsc_d[:])
            ue = constp.tile([128, 2, BL], f32)
            nc.sync.dma_start(ue[:], ue_d[:])

            outh = constp.tile([128, 16, BL], f32)
            nc.vector.memset(outh[64:128, 15, :], 0.0)

            # ---- horizontal convs, heights paired (2k-1, 2k) ----
            for k in range(1, 16):
                i1 = 2 * k - 1
                T = 33 - 2 * k          # positions computed (incl. 1 junk for i2)
                nj = 2 * k              # shifts j = 0..i2-1
                off = k * (k - 1)
                wt = wtp.tile([128, nj, 2, 128], f32, tag="wt")
                nc.sync.dma_start(wt[:], whm_d[:, off:off + nj])
                bpc = min(BL, 512 // T)
                nch = (BL + bpc - 1) // bpc
                for c in range(nch):
                    b0 = c * bpc
                    nb = min(bpc, BL - b0)
                    N = nb * T
                    ps = psconv.tile([128, 512], f32, tag="cv")
                    for j in range(nj):
                        for dc in range(2):
                            nc.tensor.matmul(
                                ps[:, 0:N], wt[:, j, dc, :],
                                xt[:, dc, b0:b0 + nb, j:j + T],
                                start=(j == 0 and dc == 0),
                                stop=(j == nj - 1 and dc == 1))
                    v3 = ps[:, 0:N].rearrange("p (b t) -> p b t", t=T)
                    tmp = tmpp.tile([128, 64], f32, tag="tmp")
                    nc.vector.reduce_max(tmp[0:64, 0:nb], v3[0:64], axis=X)
                    nc.vector.reduce_max(tmp[64:128, 0:nb],
                                         v3[64:128, :, 0:T - 1], axis=X)
                    nc.scalar.activation(outh[:, k - 1, b0:b0 + nb],
                                         tmp[:, 0:nb], Relu,
                                         bias=bh_t[:, k - 1:k])

            # ---- height 31 alone (T=1: no pooling) ----
            ws = wtp.tile([128, 31, 2, 64], f32, tag="wt")
            nc.sync.dma_start(ws[:], whs_d[:])
            ps31 = psconv.tile([128, 64], f32, tag="cv")
            for j in range(31):
                for dc in range(2):
                    nc.tensor.matmul(ps31[0:64, 0:BL], ws[:, j, dc, :],
                                     xt[:, dc, :, j:j + 1],
                                     start=(j == 0 and dc == 0),
                                     stop=(j == 30 and dc == 1))
            nc.scalar.activation(outh[0:64, 15, :], ps31[0:64, 0:BL], Relu,
                                 bias=bh_t[0:64, 15:16])

            # ---- z = relu(x.Mfold + out_h @ W1h^T + b1') ----
            psz = [psfc.tile([128, BL], f32, tag="fc", name=f"psz{kc}")
                   for kc in range(2)]
            for lc in range(4):
                ln = 8 if lc < 3 else 7
                mft = mfp.tile([128, ln, 2, 2, 128], f32, tag="mf")
                nc.sync.dma_start(mft[:], mf_d[:, lc * 8:lc * 8 + ln])
                for li in range(ln):
                    l = lc * 8 + li
                    for dc in range(2):
                        for kc in range(2):
                            nc.tensor.matmul(psz[kc], mft[:, li, dc, kc, :],
                                             xt[:, dc, :, l],
                                             start=(l == 0 and dc == 0),
                                             stop=False)
            for pair in range(16):
                for kc in range(2):
                    nc.tensor.matmul(psz[kc], w1h[:, pair, kc, :],
                                     outh[:, pair, :], start=False,
                                     stop=(pair == 15))
            z_sb = constp.tile([128, 2, BL], f32)
            for kc in range(2):
                nc.scalar.activation(z_sb[:, kc], psz[kc], Relu,
                                     bias=b1_t[:, kc:kc + 1])

            # ---- seq_output = relu([z; ue] @ W2^T + b2) ----
            s_sb = constp.tile([128, 4, BL], f32)
            for hc in range(4):
                pss = psfc.tile([128, BL], f32, tag="fc")
                for fc in range(4):
                    rhs = z_sb[:, fc, :] if fc < 2 else ue[:, fc - 2, :]
                    nc.tensor.matmul(pss, w2[:, fc, hc, :], rhs,
                                     start=(fc == 0), stop=(fc == 3))
                nc.scalar.activation(s_sb[:, hc], pss, Relu,
                                     bias=b2_t[:, hc:hc + 1])
            nc.scalar.dma_start(s_out_d[:], s_sb[:])

            # ---- AllGather seq_output across the 8 cores ----
            s_loc = dramp.tile([128, 4, BL], f32)
            s_all = dramp.tile([NCORES, 128, 4, BL], f32)
            nc.gpsimd.dma_start(s_loc[:], s_sb[:])
            nc.gpsimd.collective_compute(
                "AllGather", mybir.AluOpType.bypass,
                replica_groups=[list(range(NCORES))],
                ins=[s_loc.opt()], outs=[s_all.opt()])
            sfull = constp.tile([128, 4, NCORES, BL], f32)
            nc.scalar.dma_start(sfull[:],
                                s_all[:].rearrange("c p h b -> p h c b"))

            # ---- hidden_seq = item_seq @ Wsc^T + bsc (fills AllGather gap) ----
            for mc in range(4):
                for bb in range(4):
                    b0 = bb * 16
                    ph = psout.tile([128, 496], f32, tag="out")
                    for dc in range(2):
                        nc.tensor.matmul(ph[:, :], wsc[:, dc, mc, :],
                                         xt[:, dc, b0:b0 + 16, 0:L],
                                         start=(dc == 0), stop=(dc == 1))
                    hs = evacp.tile([128, 496], f32, tag="ev")
                    nc.scalar.activation(hs[:], ph[:], Ident,
                                         bias=bsc_t[:, mc:mc + 1])
                    nc.scalar.dma_start(
                        hs_out_d[:, mc, b0:b0 + 16, :],
                        hs[:].rearrange("p (b t) -> p b t", t=L))

            # ---- logit = seq_output @ Wp^T + bp (vocab-sharded) ----
            for vc in range(VCH):
                wp = wpp.tile([128, 4, 128], f32, tag="wp")
                nc.scalar.dma_start(wp[:], wp_d[:, :, vc * 128:(vc + 1) * 128])
                pl = psout.tile([128, 512], f32, tag="out")
                for hc in range(4):
                    nc.tensor.matmul(pl, wp[:, hc, :], sfull[:, hc],
                                     start=(hc == 0), stop=(hc == 3))
                lg = evacp.tile([128, 512], f32, tag="ev")
                nc.scalar.activation(lg[:], pl[:], Ident,
                                     bias=bp_t[:, vc:vc + 1])
                nc.scalar.dma_start(lg_out_d[vc], lg[:])

    nc.compile()
    return nc


def _get_program():
    if "nc" not in _CACHE:
        _CACHE["nc"] = _build_program()
    return _CACHE["nc"]


def _prep_shared(item_seq, user, user_emb_table, Wv, bv, Wh, bh, W1, b1, W2,
                 b2, Wsc, bsc):
    """Host-side packing of all core-replicated tensors."""
    f = np.float32
    # horizontal conv weights, pair-packed: [d_part, jslot, dchunk, m]
    WhT = np.ascontiguousarray(Wh.transpose(3, 0, 2, 1))       # [256,i,j,f]
    WhTv = WhT.reshape(2, 128, L, L, NH)                       # [dc,p,i,j,f]
    whm = np.zeros((128, 240, 2, 128), f)
    for k in range(1, 16):
        i1, i2 = 2 * k - 1, 2 * k
        off = k * (k - 1)
        whm[:, off:off + i1, :, 0:64] = \
            WhTv[:, :, i1 - 1, 0:i1, :].transpose(1, 2, 0, 3)
        whm[:, off:off + i2, :, 64:128] = \
            WhTv[:, :, i2 - 1, 0:i2, :].transpose(1, 2, 0, 3)
    whs = np.ascontiguousarray(WhTv[:, :, 30, :, :].transpose(1, 2, 0, 3))

    bh_t = np.zeros((128, 16), f)
    for k in range(1, 16):
        bh_t[0:64, k - 1] = bh[2 * k - 2]
        bh_t[64:128, k - 1] = bh[2 * k - 1]
    bh_t[0:64, 15] = bh[30]

    # fold vertical conv into fc1
    W1v = W1[:, :NV * D].astype(np.float64).reshape(D, NV, D)
    M0 = np.tensordot(W1v, Wv.astype(np.float64), axes=([1], [0]))  # [k,d,l]
    M0 = M0.transpose(0, 2, 1)                                      # [k,l,d]
    mfl = np.ascontiguousarray(
        M0.transpose(2, 1, 0).reshape(2, 128, L, 2, 128)
        .transpose(1, 2, 0, 3, 4)).astype(f)                 # [p,l,dc,kc,kk]
    b1p = (b1.astype(np.float64)
           + W1v.sum(axis=2) @ bv.astype(np.float64)).astype(f)
    b1p_t = np.ascontiguousarray(b1p.reshape(2, 128).T)      # [p, kc]

    W1h = W1[:, NV * D:].reshape(D, L, NH)                   # [k, i, f]
    P16 = np.zeros((128, 16, D), f)
    for k in range(1, 16):
        P16[0:64, k - 1, :] = W1h[:, 2 * k - 2, :].T
        P16[64:128, k - 1, :] = W1h[:, 2 * k - 1, :].T
    P16[0:64, 15, :] = W1h[:, 30, :].T
    w1ht = np.ascontiguousarray(P16.reshape(128, 16, 2, 128))

    w2t = np.ascontiguousarray(
        W2.T.reshape(4, 128, 4, 128).transpose(1, 0, 2, 3))
    b2r = np.ascontiguousarray(b2.reshape(4, 128).T)
    wsct = np.ascontiguousarray(
        Wsc.T.reshape(2, 128, 4, 128).transpose(1, 0, 2, 3))
    bsc_t = np.ascontiguousarray(bsc.reshape(4, 128).T)

    return dict(whm=whm, whs=whs, bh_t=bh_t, mfl=mfl, b1p=b1p_t, w1ht=w1ht,
                w2t=w2t, b2r=b2r, wsct=wsct, bsc_t=bsc_t)


def kernel(user, item_seq, user_emb_table, Wv, bv, Wh, bh, W1, b1, W2, b2,
           Wp, bp, Wsc, bsc):
    import sys
    if "/opt/trn_rl_repo" not in sys.path:
        sys.path.insert(0, "/opt/trn_rl_repo")
    from concourse.bass_utils import run_bass_kernel_spmd

    f = np.float32
    user = np.asarray(user)
    item_seq = np.asarray(item_seq, f)
    shared = _prep_shared(item_seq, user, np.asarray(user_emb_table, f),
                          np.asarray(Wv, f), np.asarray(bv, f),
                          np.asarray(Wh, f), np.asarray(bh, f),
                          np.asarray(W1, f), np.asarray(b1, f),
                          np.asarray(W2, f), np.asarray(b2, f),
                          np.asarray(Wsc, f), np.asarray(bsc, f))
    ue_all = np.asarray(user_emb_table, f)[user]              # [B, D]
    Wp = np.asarray(Wp, f)
    bp = np.asarray(bp, f)

    in_maps = []
    for c in range(NCORES):
        sl = slice(c * BL, (c + 1) * BL)
        x = item_seq[sl]                                      # [64, 31, 256]
        xpad = np.zeros((BL, 32, D), f)
        xpad[:, :L] = x
        xT = np.ascontiguousarray(
            xpad.transpose(2, 0, 1).reshape(2, 128, BL, 32))
        ueT = np.ascontiguousarray(
            ue_all[sl].T.reshape(2, 128, BL).transpose(1, 0, 2))
        Wp_sh = Wp[c * VS:(c + 1) * VS]
        Wp_pad = np.zeros((VPAD, H), f)
        Wp_pad[:VS] = Wp_sh
        wpt = np.ascontiguousarray(
            Wp_pad.T.reshape(4, 128, VPAD).transpose(1, 0, 2))
        bp_pad = np.zeros(VPAD, f)
        bp_pad[:VS] = bp[c * VS:(c + 1) * VS]
        bp_t = np.ascontiguousarray(bp_pad.reshape(VCH, 128).T)
        m = dict(shared)
        m.update(xT=xT, ueT=ueT, wpt=wpt, bp_t=bp_t)
        in_maps.append(m)

    nc = _get_program()
    global _LAST_IN_MAPS
    _LAST_IN_MAPS = in_maps
    res = run_bass_kernel_spmd(nc, in_maps, list(range(NCORES)))
    results = res.results

    logit = np.empty((B, V), f)
    hidden_seq = np.empty((B, L, H), f)
    seq_output = np.empty((B, H), f)
    for c in range(NCORES):
        r = results[c]
        lg = r["logit_out"].reshape(VPAD, 512)[:VS]           # [v_loc, b_glob]
        logit[:, c * VS:(c + 1) * VS] = lg.T
        hs = r["hseq_out"]                                    # [p, mc, b, l]
        hidden_seq[c * BL:(c + 1) * BL] = \
            hs.transpose(2, 3, 1, 0).reshape(BL, L, H)
        so = r["s_out"]                                       # [p, hc, b]
        seq_output[c * BL:(c + 1) * BL] = \
            so.transpose(2, 1, 0).reshape(BL, H)
    return logit, hidden_seq, seq_output
